# revision 37
# baseline (speedup 1.0000x reference)
"""3x3 conv (256->256, stride 1, pad 1) as implicit GEMM on 8 TRN2 NeuronCores.

Data-parallel over batch: 32 images -> 4 per core; weight/bias replicated.

Per core, per image: x is resident in SBUF as two [128, 56, 58] channel
tiles in bf16 (zero columns at w=0 and w=57 provide the horizontal conv
padding). For each output row-tile of 8 rows and each of 2 output-channel
tiles, 18 matmuls (9 conv taps x 2 input-channel tiles) accumulate into a
PSUM tile [128, 8, 56]. bf16 operands stream at 1 col/cycle and get fast
(FWL) weight loads that hide under the previous matmul, so the PE runs at
its 448-cycle/matmul streaming floor. Padding is handled by clipping each
tap's row range via 3D access patterns; the center tap runs first with
start=True so every PSUM element's first write is an overwrite. Bias is
fused into the PSUM->SBUF eviction on the scalar engine.

Pipeline structure (the matmul stream itself runs at the PE streaming
floor, ~189ns per 448-column matmul; everything else hides under it):
  - x images are loaded in row chunks (DMA to contiguous staging on the
    scalar ring, DVE restride into the padded tile); subtile dependency
    tracking lets row-tile matmuls start as soon as their rows are
    resident, cutting start-of-kernel DMA latency out of the PE timeline.
  - loads are issued one image ahead of compute (2 SBUF slots).
  - weights: taps 0-4 as one large sync-ring DMA (large transfers ramp
    the DMA queues ~3x faster than small pieces), taps 5-8 interleaved
    with the early x chunks on the scalar ring, each landing just before
    the matmul stream consumes it (per-tile tap order matches arrival).
  - output evictions alternate between the sync and scalar (HWDGE)
    rings; the final tile evicts in two halves so the last HBM write
    drains sooner (the gpsimd SWDGE ring is ~10x slower to drain).
  - warm-up matmuls on a zero tile bridge the PE from queue-open until
    the first real matmul so the HAM clock-gate is at 8/8 (2.4 GHz, not
    the cold 1.2 GHz) when the real stream starts.

The host wrapper pre-reorders the weight to [c, tap, ci, co, o] so lhsT
tiles ([c_in 128, c_out 128] per tap) DMA straight into SBUF.
"""

from contextlib import ExitStack

import numpy as np

import os

import concourse.bass as bass  # noqa: F401  (bass types used via tc/nc)
import concourse.tile as tile
from concourse import bacc, mybir
from concourse.bass_utils import run_bass_kernel_spmd

N_CORES = 8
N_TOTAL = 32
N_PER = N_TOTAL // N_CORES  # 4 images per core
C = 256
H = W = 56
RT = 8          # output rows per PSUM tile -> 8*56 = 448 <= 512 (one bank)
NRT = H // RT   # 7 row tiles
F32 = mybir.dt.float32
# compute dtype for the matmul operands (storage + PE streaming format)
_MM_DT_NAME = os.environ.get("CONV_MM_DTYPE", "bfloat16")
MM_DT = getattr(mybir.dt, _MM_DT_NAME)
MM_NP = mybir.dt.np(MM_DT)
N_WARM = int(os.environ.get("CONV_WARM_MMS", "15"))
# row chunks per image load; each restride unlocks the row-tiles it covers.
# The first chunk is just the rows row-tile 0 needs so matmuls start ASAP;
# later chunks are bigger (fewer DMAs ramp the queues faster).
CHUNKS = ((0, 9), (9, 28), (28, 56))

_CACHE = {}


def _build():
    nc = bacc.Bacc(
        "TRN2", target_bir_lowering=False, debug=False, num_devices=N_CORES
    )
    # channel dim pre-split [ci, 128] so one DMA can carry both halves
    # of a row chunk (transposed 4D access pattern)
    xs = nc.dram_tensor(
        "xs", [N_PER, 2, 128, H, W], MM_DT, kind="ExternalInput"
    ).ap()
    wt = nc.dram_tensor(
        "wt", [128, 9, 2, 2, 128], MM_DT, kind="ExternalInput"
    ).ap()
    b2 = nc.dram_tensor("b2", [128, 2], F32, kind="ExternalInput").ap()
    out = nc.dram_tensor(
        "out", [N_PER, C, H, W], F32, kind="ExternalOutput"
    ).ap()

    # Accumulation order: center tap (full coverage) first so its
    # start=True write touches every element of the PSUM tile; after
    # that, taps in the order their DMA pieces land (sync ring first,
    # slow gpsimd/SWDGE pieces last) so the first tiles never stall.
    order = []
    for tap in (4, 0, 1, 2, 3, 5, 6, 7, 8):
        for ci in (0, 1):
            order.append((tap // 3, tap % 3, ci))

    with tile.TileContext(nc) as tc, ExitStack() as ctx:
        wpool = ctx.enter_context(tc.tile_pool(name="w", bufs=1))
        spool = ctx.enter_context(tc.tile_pool(name="s", bufs=4))
        bpool = ctx.enter_context(tc.tile_pool(name="b", bufs=1))
        xpool = ctx.enter_context(tc.tile_pool(name="x", bufs=1))
        opool = ctx.enter_context(tc.tile_pool(name="o", bufs=6))
        ppool = ctx.enter_context(tc.tile_pool(name="p", bufs=5, space="PSUM"))
        warmp = ctx.enter_context(
            tc.tile_pool(name="wm", bufs=1, space="PSUM")
        )

        # Warm-up: matmuls over a zero scratch tile (PSUM bank never
        # read). The gpsimd memset is the earliest-starting engine op, so
        # these issue almost as soon as the PE queue opens and keep the
        # HAM clock-gate ramping while the first DMAs are in flight.
        zz = bpool.tile([128, 448], MM_DT)
        nc.gpsimd.memset(zz[:], 0.0)
        if N_WARM:
            wps = warmp.tile([128, 448], F32)
            for _ in range(N_WARM):
                nc.tensor.matmul(
                    wps[:], zz[:, 0:128], zz[:], start=True, stop=True
                )

        # Pad-column zero source (f32; DVE converts on copy).
        z_sb = bpool.tile([128, H, 1], F32)
        nc.vector.memset(z_sb[:], 0.0)

        # Weights split across all four DMA rings; the first x chunk goes
        # ahead of the weight pieces on sync/scalar and the centre tap
        # (first matmul of every accumulation group) rides alone on the
        # vector ring, so row-tile 0 can start within ~1us of ring start.
        w_sb = wpool.tile([128, 9, 2, 2, 128], MM_DT)
        b_sb = bpool.tile([128, 2], F32)

        def issue_w():
            # Taps 0-4 (consumed first; includes the centre tap) as one
            # large sync-ring transfer; taps 5-8 ride the scalar ring in
            # two pieces interleaved with the early x chunks so each tap
            # lands just before the matmul stream consumes it.
            nc.sync.dma_start(w_sb[:, 0:5], wt[:, 0:5])
            nc.gpsimd.dma_start(b_sb[:], b2[:, :])

        # persistent, manually double-buffered x tiles: [slot][ci]
        x_tiles = []
        for slot in range(2):
            row = []
            for ci in range(2):
                t = xpool.tile(
                    [128, H, W + 2], MM_DT, tag=f"x{slot}{ci}"
                )
                nc.vector.tensor_copy(t[:, :, 0:1], z_sb[:])
                nc.vector.tensor_copy(t[:, :, W + 1 : W + 2], z_sb[:])
                row.append(t)
            x_tiles.append(row)

        def issue_loads(n, chunks=CHUNKS):
            # one DMA per row chunk carrying both channel halves (pays
            # the per-DMA queue latency once); all x rides the scalar
            # ring (the sync ring carries the weight block).
            x_sb = x_tiles[n % 2]
            for k, (r0, r1) in enumerate(chunks):
                stg = spool.tile(
                    [128, 2, r1 - r0, W], MM_DT, tag=f"s{k % 2}"
                )
                nc.scalar.dma_start(
                    stg[:],
                    xs[n, :, :, r0:r1, :].transpose((1, 0, 2, 3)),
                )
                for ci in range(2):
                    nc.vector.tensor_copy(
                        x_sb[ci][:, r0:r1, 1 : W + 1], stg[:, ci]
                    )

        issue_w()
        issue_loads(0, chunks=CHUNKS[:1])
        nc.scalar.dma_start(w_sb[:, 5:7], wt[:, 5:7])
        issue_loads(0, chunks=CHUNKS[1:2])
        nc.scalar.dma_start(w_sb[:, 7:9], wt[:, 7:9])
        issue_loads(0, chunks=CHUNKS[2:])
        out_rings = (nc.sync, nc.scalar)
        n_out = 0
        for n in range(N_PER):
            if n + 1 < N_PER:
                issue_loads(n + 1)
            x_sb = x_tiles[n % 2]
            for rt in range(NRT):
                h0 = rt * RT
                for co in range(2):
                    ps = ppool.tile([128, RT, W], F32)
                    for i, (kh, kw, ci) in enumerate(order):
                        dh = kh - 1
                        r0 = max(h0, -dh)
                        r1 = min(h0 + RT, H - dh)
                        lhsT = w_sb[:, kh * 3 + kw, ci, co, :]
                        rhs = x_sb[ci][:, r0 + dh : r1 + dh, kw : kw + W]
                        nc.tensor.matmul(
                            ps[:, r0 - h0 : r1 - h0, :],
                            lhsT,
                            rhs,
                            start=(i == 0),
                            stop=(i == len(order) - 1),
                        )
                    o_sb = opool.tile([128, RT, W], F32)
                    last = n == N_PER - 1 and rt == NRT - 1 and co == 1
                    # the very last tile evicts in two halves on separate
                    # rings so the final HBM write drains ~1.5us sooner
                    halves = ((0, 4), (4, RT)) if last else ((0, RT),)
                    for g0, g1 in halves:
                        nc.scalar.activation(
                            o_sb[:, g0:g1],
                            ps[:, g0:g1],
                            mybir.ActivationFunctionType.Identity,
                            bias=b_sb[:, co : co + 1],
                        )
                    for g0, g1 in halves:
                        out_rings[n_out % 2].dma_start(
                            out[
                                n,
                                co * 128 : (co + 1) * 128,
                                h0 + g0 : h0 + g1,
                                :,
                            ],
                            o_sb[:, g0:g1],
                        )
                        n_out += 1
    nc.compile()
    return nc


def _get_nc():
    if "nc" not in _CACHE:
        _CACHE["nc"] = _build()
    return _CACHE["nc"]


def _in_maps(x, weight, bias):
    x = np.ascontiguousarray(
        np.asarray(x, dtype=np.float32).astype(MM_NP)
    ).reshape(N_TOTAL, 2, 128, H, W)
    weight = np.asarray(weight, dtype=np.float32)
    bias = np.asarray(bias, dtype=np.float32)
    # weight[co*128+o, (ci*128+c)*9 + (kh*3+kw)] -> wt[c, tap, ci, co, o]
    wt = np.ascontiguousarray(
        weight.reshape(2, 128, 2, 128, 9).transpose(3, 4, 2, 0, 1).astype(MM_NP)
    )
    b2 = np.ascontiguousarray(bias.reshape(2, 128).T)
    return [
        {"xs": x[i * N_PER : (i + 1) * N_PER], "wt": wt, "b2": b2}
        for i in range(N_CORES)
    ]


def _run(x, weight, bias, trace=False):
    res = run_bass_kernel_spmd(
        _get_nc(),
        _in_maps(x, weight, bias),
        core_ids=list(range(N_CORES)),
        trace=trace,
    )
    out = np.concatenate(
        [res.results[i]["out"] for i in range(N_CORES)], axis=0
    )
    return out, res


def kernel(x, weight, bias):
    out, _ = _run(x, weight, bias, trace=False)
    return out


def run_profiled(x, weight, bias):
    out, res = _run(x, weight, bias, trace=True)
    return out, res.exec_time_ns


# revision 40
# speedup vs baseline: 1.0009x; 1.0009x over previous
"""3x3 conv (256->256, stride 1, pad 1) as implicit GEMM on 8 TRN2 NeuronCores.

Data-parallel over batch: 32 images -> 4 per core; weight/bias replicated.

Per core, per image: x is resident in SBUF as two [128, 56, 58] channel
tiles in bf16 (zero columns at w=0 and w=57 provide the horizontal conv
padding). For each output row-tile of 8 rows and each of 2 output-channel
tiles, 18 matmuls (9 conv taps x 2 input-channel tiles) accumulate into a
PSUM tile [128, 8, 56]. bf16 operands stream at 1 col/cycle and get fast
(FWL) weight loads that hide under the previous matmul, so the PE runs at
its 448-cycle/matmul streaming floor. Padding is handled by clipping each
tap's row range via 3D access patterns; the center tap runs first with
start=True so every PSUM element's first write is an overwrite. Bias is
fused into the PSUM->SBUF eviction on the scalar engine.

Pipeline structure (the matmul stream itself runs at the PE streaming
floor, ~189ns per 448-column matmul; everything else hides under it):
  - x images are loaded in row chunks (DMA to contiguous staging on the
    scalar ring, DVE restride into the padded tile); subtile dependency
    tracking lets row-tile matmuls start as soon as their rows are
    resident, cutting start-of-kernel DMA latency out of the PE timeline.
  - loads are issued one image ahead of compute (2 SBUF slots).
  - weights: taps 0-4 as one large sync-ring DMA (large transfers ramp
    the DMA queues ~3x faster than small pieces), taps 5-8 interleaved
    with the early x chunks on the scalar ring, each landing just before
    the matmul stream consumes it (per-tile tap order matches arrival).
  - output evictions alternate between the sync and scalar (HWDGE)
    rings; the final tile evicts in two halves so the last HBM write
    drains sooner (the gpsimd SWDGE ring is ~10x slower to drain).
  - warm-up matmuls on a zero tile bridge the PE from queue-open until
    the first real matmul so the HAM clock-gate is at 8/8 (2.4 GHz, not
    the cold 1.2 GHz) when the real stream starts.

The host wrapper pre-reorders the weight to [c, tap, ci, co, o] so lhsT
tiles ([c_in 128, c_out 128] per tap) DMA straight into SBUF.
"""

from contextlib import ExitStack

import numpy as np

import os

import concourse.bass as bass  # noqa: F401  (bass types used via tc/nc)
import concourse.tile as tile
from concourse import bacc, mybir
from concourse.bass_utils import run_bass_kernel_spmd

N_CORES = 8
N_TOTAL = 32
N_PER = N_TOTAL // N_CORES  # 4 images per core
C = 256
H = W = 56
RT = 8          # output rows per PSUM tile -> 8*56 = 448 <= 512 (one bank)
NRT = H // RT   # 7 row tiles
F32 = mybir.dt.float32
# compute dtype for the matmul operands (storage + PE streaming format)
_MM_DT_NAME = os.environ.get("CONV_MM_DTYPE", "bfloat16")
MM_DT = getattr(mybir.dt, _MM_DT_NAME)
MM_NP = mybir.dt.np(MM_DT)
N_WARM = int(os.environ.get("CONV_WARM_MMS", "15"))
# row chunks per image load; each restride unlocks the row-tiles it covers.
# The first chunk is just the rows row-tile 0 needs so matmuls start ASAP;
# later chunks are bigger (fewer DMAs ramp the queues faster).
CHUNKS = ((0, 9), (9, 28), (28, 56))

_CACHE = {}


def _build():
    nc = bacc.Bacc(
        "TRN2", target_bir_lowering=False, debug=False, num_devices=N_CORES
    )
    xs = nc.dram_tensor(
        "xs", [N_PER, C, H, W], MM_DT, kind="ExternalInput"
    ).ap()
    wt = nc.dram_tensor(
        "wt", [128, 9, 2, 2, 128], MM_DT, kind="ExternalInput"
    ).ap()
    b2 = nc.dram_tensor("b2", [128, 2], F32, kind="ExternalInput").ap()
    out = nc.dram_tensor(
        "out", [N_PER, C, H, W], F32, kind="ExternalOutput"
    ).ap()

    # Accumulation order: center tap (full coverage) first so its
    # start=True write touches every element of the PSUM tile; after
    # that, taps in the order their DMA pieces land (sync ring first,
    # slow gpsimd/SWDGE pieces last) so the first tiles never stall.
    order = []
    for tap in (4, 0, 1, 2, 3, 5, 6, 7, 8):
        for ci in (0, 1):
            order.append((tap // 3, tap % 3, ci))

    with tile.TileContext(nc) as tc, ExitStack() as ctx:
        wpool = ctx.enter_context(tc.tile_pool(name="w", bufs=1))
        spool = ctx.enter_context(tc.tile_pool(name="s", bufs=4))
        bpool = ctx.enter_context(tc.tile_pool(name="b", bufs=1))
        xpool = ctx.enter_context(tc.tile_pool(name="x", bufs=1))
        opool = ctx.enter_context(tc.tile_pool(name="o", bufs=6))
        ppool = ctx.enter_context(tc.tile_pool(name="p", bufs=5, space="PSUM"))
        warmp = ctx.enter_context(
            tc.tile_pool(name="wm", bufs=1, space="PSUM")
        )

        # Warm-up: matmuls over a zero scratch tile (PSUM bank never
        # read). The gpsimd memset is the earliest-starting engine op, so
        # these issue almost as soon as the PE queue opens and keep the
        # HAM clock-gate ramping while the first DMAs are in flight.
        zz = bpool.tile([128, 448], MM_DT)
        nc.gpsimd.memset(zz[:], 0.0)
        if N_WARM:
            wps = warmp.tile([128, 448], F32)
            for _ in range(N_WARM):
                nc.tensor.matmul(
                    wps[:], zz[:, 0:128], zz[:], start=True, stop=True
                )

        # Pad-column zero source (f32; DVE converts on copy).
        z_sb = bpool.tile([128, H, 1], F32)
        nc.vector.memset(z_sb[:], 0.0)

        # Weights split across all four DMA rings; the first x chunk goes
        # ahead of the weight pieces on sync/scalar and the centre tap
        # (first matmul of every accumulation group) rides alone on the
        # vector ring, so row-tile 0 can start within ~1us of ring start.
        w_sb = wpool.tile([128, 9, 2, 2, 128], MM_DT)
        b_sb = bpool.tile([128, 2], F32)

        def issue_w():
            # Taps 0-4 (consumed first; includes the centre tap) as one
            # large sync-ring transfer; taps 5-8 ride the scalar ring in
            # two pieces interleaved with the early x chunks so each tap
            # lands just before the matmul stream consumes it.
            nc.sync.dma_start(w_sb[:, 0:5], wt[:, 0:5])
            nc.gpsimd.dma_start(b_sb[:], b2[:, :])

        # persistent, manually double-buffered x tiles: [slot][ci]
        x_tiles = []
        for slot in range(2):
            row = []
            for ci in range(2):
                t = xpool.tile(
                    [128, H, W + 2], MM_DT, tag=f"x{slot}{ci}"
                )
                nc.vector.tensor_copy(t[:, :, 0:1], z_sb[:])
                nc.vector.tensor_copy(t[:, :, W + 1 : W + 2], z_sb[:])
                row.append(t)
            x_tiles.append(row)

        def issue_loads(n, chunks=CHUNKS):
            # chunk-major so the first rows of both channel halves land
            # before any deeper rows; all x rides the scalar ring (the
            # sync ring carries the weight block).
            x_sb = x_tiles[n % 2]
            for k, (r0, r1) in enumerate(chunks):
                for ci in range(2):
                    stg = spool.tile(
                        [128, r1 - r0, W], MM_DT, tag=f"s{ci}{k % 2}"
                    )
                    nc.scalar.dma_start(
                        stg[:], xs[n, ci * 128 : (ci + 1) * 128, r0:r1, :]
                    )
                    nc.vector.tensor_copy(
                        x_sb[ci][:, r0:r1, 1 : W + 1], stg[:]
                    )

        issue_w()
        issue_loads(0, chunks=CHUNKS[:1])
        nc.scalar.dma_start(w_sb[:, 5:7], wt[:, 5:7])
        issue_loads(0, chunks=CHUNKS[1:2])
        nc.scalar.dma_start(w_sb[:, 7:9], wt[:, 7:9])
        issue_loads(0, chunks=CHUNKS[2:])
        out_rings = (nc.sync, nc.scalar)
        n_out = 0
        for n in range(N_PER):
            if n + 1 < N_PER:
                issue_loads(n + 1)
            x_sb = x_tiles[n % 2]
            for rt in range(NRT):
                h0 = rt * RT
                for co in range(2):
                    ps = ppool.tile([128, RT, W], F32)
                    for i, (kh, kw, ci) in enumerate(order):
                        dh = kh - 1
                        r0 = max(h0, -dh)
                        r1 = min(h0 + RT, H - dh)
                        lhsT = w_sb[:, kh * 3 + kw, ci, co, :]
                        rhs = x_sb[ci][:, r0 + dh : r1 + dh, kw : kw + W]
                        nc.tensor.matmul(
                            ps[:, r0 - h0 : r1 - h0, :],
                            lhsT,
                            rhs,
                            start=(i == 0),
                            stop=(i == len(order) - 1),
                        )
                    o_sb = opool.tile([128, RT, W], F32)
                    last = n == N_PER - 1 and rt == NRT - 1 and co == 1
                    # the very last tile evicts in two halves on separate
                    # rings so the final HBM write drains ~1.5us sooner
                    halves = ((0, 4), (4, RT)) if last else ((0, RT),)
                    for g0, g1 in halves:
                        nc.scalar.activation(
                            o_sb[:, g0:g1],
                            ps[:, g0:g1],
                            mybir.ActivationFunctionType.Identity,
                            bias=b_sb[:, co : co + 1],
                        )
                    for g0, g1 in halves:
                        out_rings[n_out % 2].dma_start(
                            out[
                                n,
                                co * 128 : (co + 1) * 128,
                                h0 + g0 : h0 + g1,
                                :,
                            ],
                            o_sb[:, g0:g1],
                        )
                        n_out += 1
    nc.compile()
    return nc


def _get_nc():
    if "nc" not in _CACHE:
        _CACHE["nc"] = _build()
    return _CACHE["nc"]


def _in_maps(x, weight, bias):
    x = np.ascontiguousarray(np.asarray(x, dtype=np.float32).astype(MM_NP))
    weight = np.asarray(weight, dtype=np.float32)
    bias = np.asarray(bias, dtype=np.float32)
    # weight[co*128+o, (ci*128+c)*9 + (kh*3+kw)] -> wt[c, tap, ci, co, o]
    wt = np.ascontiguousarray(
        weight.reshape(2, 128, 2, 128, 9).transpose(3, 4, 2, 0, 1).astype(MM_NP)
    )
    b2 = np.ascontiguousarray(bias.reshape(2, 128).T)
    return [
        {"xs": x[i * N_PER : (i + 1) * N_PER], "wt": wt, "b2": b2}
        for i in range(N_CORES)
    ]


def _run(x, weight, bias, trace=False):
    res = run_bass_kernel_spmd(
        _get_nc(),
        _in_maps(x, weight, bias),
        core_ids=list(range(N_CORES)),
        trace=trace,
    )
    out = np.concatenate(
        [res.results[i]["out"] for i in range(N_CORES)], axis=0
    )
    return out, res


def kernel(x, weight, bias):
    out, _ = _run(x, weight, bias, trace=False)
    return out


def run_profiled(x, weight, bias):
    out, res = _run(x, weight, bias, trace=True)
    return out, res.exec_time_ns
